# revision 43
# baseline (speedup 1.0000x reference)
"""DetectionLoss Trainium2 kernel.

Host side (mirrors the reference, which also does this on CPU): per-image
Hungarian matching on the L1 box cost, plus gathering the 30 matched
pred/target rows per image that the matching selects.  Device side
(Bass/Tile, 8 NeuronCores, data-parallel over the batch — 2 images per
core): all loss arithmetic — the full (B,N) objectness BCE scan (strided
DMA of the objectness column), GIoU over matched pairs, focal
classification loss — reduced to per-core partial sums.  Host sums the 8
partial-sum vectors and forms the three scalar losses.
"""

import numpy as np

NUM_CLASSES = 80
ALPHA, GAMMA = 0.25, 2.0
EPS = 1e-07

B, N, T, CH = 16, 8400, 30, 85  # batch, anchors, targets, channels (5+C)
NCORES = 8
BL = B // NCORES          # images per core
P = BL * T                # matched pairs per core (partition dim)
ROWS_P = 120              # partitions for the obj scan (8400 = 120*70)
ROWS_F = N // ROWS_P      # 70 rows per partition
NCOL = 8                  # partials columns


# ---------------------------------------------------------------- host match
def _lsa(cost):
    """Min-cost rectangular linear sum assignment (verbatim from reference)."""
    cost = np.asarray(cost, dtype=np.float64)
    transposed = cost.shape[0] > cost.shape[1]
    C = cost.T if transposed else cost
    n, m = C.shape
    u = np.zeros(n + 1)
    v = np.zeros(m + 1)
    p = np.zeros(m + 1, dtype=np.int64)
    way = np.zeros(m + 1, dtype=np.int64)
    Cp = np.zeros((n + 1, m + 1))
    Cp[1:, 1:] = C
    for i in range(1, n + 1):
        p[0] = i
        j0 = 0
        minv = np.full(m + 1, np.inf)
        used = np.zeros(m + 1, dtype=bool)
        while True:
            used[j0] = True
            i0 = p[j0]
            js = np.nonzero(~used)[0]
            cur = Cp[i0, js] - u[i0] - v[js]
            better = cur < minv[js]
            minv[js] = np.where(better, cur, minv[js])
            way[js[better]] = j0
            k = int(np.argmin(minv[js]))
            delta = minv[js][k]
            j1 = int(js[k])
            u[p[used]] += delta
            v[used] -= delta
            minv[js] -= delta
            j0 = j1
            if p[j0] == 0:
                break
        while j0:
            j1 = way[j0]
            p[j0] = p[j1]
            j0 = j1
    rows = p[1:] - 1
    cols = np.arange(m)
    mask = rows >= 0
    r, c = rows[mask], cols[mask]
    if transposed:
        r, c = c, r
    order = np.argsort(r)
    return r[order], c[order]


def _match(pred_boxes_np, target_boxes_np):
    Bn, _, _ = pred_boxes_np.shape
    Tn = target_boxes_np.shape[1]
    rows = np.zeros((Bn, Tn), dtype=np.int64)
    cols = np.zeros((Bn, Tn), dtype=np.int64)
    for b in range(Bn):
        cost = np.abs(
            pred_boxes_np[b][:, None, :] - target_boxes_np[b][None, :, :]
        ).sum(-1)
        r, c = _lsa(cost)
        rows[b], cols[b] = r, c
    return rows, cols


# ---------------------------------------------------------------- device kernel
_BUILT = None


def _build():
    """Raw-Bass (no Tile) build: three engines (sync, scalar, vector) with a
    hand-rolled semaphore plan — avoids the Tile entry barrier and exit
    butterfly so the column DMAs start right after engine bring-up.

    Raw-bass DVE ops have no implicit same-engine interlock (a distance-1
    consumer can read SBUF before the producer's write lands — verified on
    HW), so every dependent DVE pair is gated on a self-semaphore.
    """
    import concourse.bass as bass
    from concourse import mybir

    f32 = mybir.dt.float32
    AF = mybir.ActivationFunctionType
    OP = mybir.AluOpType

    nc = bass.Bass("TRN2", target_bir_lowering=False)

    preds = nc.declare_dram_parameter("preds", [BL, N, CH], f32, isOutput=False)
    # aux = [matched pred rows (85) | matched tgt rows (5) | iota (80)]
    AUXW = CH + 5 + NUM_CLASSES
    aux = nc.declare_dram_parameter("aux", [P, AUXW], f32, isOutput=False)
    out = nc.declare_dram_parameter("out", [128, NCOL], f32, isOutput=True)

    # ring split of image 1's objectness column (sync ~957 el/us since it
    # starts ~2us earlier than the scalar ring, which also carries aux)
    RA = 10
    Cc = NUM_CLASSES

    partials = nc.alloc_sbuf_tensor("k_partials", [128, NCOL], f32)
    auxt = nc.alloc_sbuf_tensor("k_auxt", [P, AUXW], f32)
    col0 = nc.alloc_sbuf_tensor("k_col0", [ROWS_P, ROWS_F, 1], f32)
    col1 = nc.alloc_sbuf_tensor("k_col1", [ROWS_P, ROWS_F, 1], f32)
    objt0 = nc.alloc_sbuf_tensor("k_objt0", [ROWS_P, ROWS_F, 1], f32)
    objt1 = nc.alloc_sbuf_tensor("k_objt1", [ROWS_P, ROWS_F, 1], f32)
    prime = nc.alloc_sbuf_tensor("k_prime", [1, 1], f32)

    def g2(name):
        return nc.alloc_sbuf_tensor("k_" + name, [P, 2], f32)

    def g1(name):
        return nc.alloc_sbuf_tensor("k_" + name, [P, 1], f32)

    def gc(name):
        return nc.alloc_sbuf_tensor("k_" + name, [P, Cc], f32)

    p1, p2, t1, t2 = g2("p1"), g2("p2"), g2("t1"), g2("t2")
    imin, imax, dd, dr = g2("imin"), g2("imax"), g2("dd"), g2("dr")
    areas, cmax, cmin, dc = g2("areas"), g2("cmax"), g2("cmin"), g2("dc")
    inter, pta, union = g1("inter"), g1("pta"), g1("union")
    ueps, runi, iou = g1("ueps"), g1("runi"), g1("iou")
    conv, cmu, ceps, rcon, term = (g1("conv"), g1("cmu"), g1("ceps"),
                                   g1("rcon"), g1("term"))
    gfin = g1("gfin")
    oh, lp, l1p, df, sel, s = (gc("oh"), gc("lp"), gc("l1p"), gc("df"),
                               gc("sel"), gc("s"))
    q12, sel2, om, sq, foc = (gc("q12"), gc("sel2"), gc("om"), gc("sq"),
                              gc("foc"))

    mpred_t = auxt[:, 0:CH]
    mtgt_t = auxt[:, CH:CH + 5]
    iota_t = auxt[:, CH + 5:AUXW]

    src_all = preds.ap().rearrange("b (p r) c -> (b p) r c", p=ROWS_P)
    src0 = src_all[0:ROWS_P, :, 4:5]
    src1 = src_all[ROWS_P:2 * ROWS_P, :, 4:5]

    s_aux = nc.alloc_semaphore("s_aux")
    s_col0 = nc.alloc_semaphore("s_col0")
    s_col1a = nc.alloc_semaphore("s_col1a")
    s_col1b = nc.alloc_semaphore("s_col1b")
    s_out = nc.alloc_semaphore("s_out")
    s_outa = nc.alloc_semaphore("s_outa")
    s_act = nc.alloc_semaphore("s_act")
    s_obj = nc.alloc_semaphore("s_obj")
    s_dve = nc.alloc_semaphore("s_dve")
    s_g = nc.alloc_semaphore("s_g")

    with nc.Block() as block:

        @block.sync
        def _(sync):
            # objectness column element-gathers on the SP HWDGE ring;
            # no waits — start as soon as the engine is up.
            with nc.allow_non_contiguous_dma(reason="strided obj column"):
                sync.dma_start(col0[:], src0).then_inc(s_col0, 16)
                sync.dma_start(col1[:, 0:RA, :],
                               src1[:, 0:RA, :]).then_inc(s_col1a, 16)
            # bulk result dump (everything except obj1's column) — issued as
            # soon as obj0 + the DVE accumulations land, transfers hidden
            # under this ring's remaining column drain
            sync.wait_ge(s_obj, 1)
            sync.wait_ge(s_dve, 1)
            sync.dma_start(out.ap()[:, 1:NCOL],
                           partials[:, 1:NCOL]).then_inc(s_outa, 16)
            sync.wait_ge(s_outa, 16)


        @block.vector
        def _(vector):
            cnt = [0]

            def step(ins):
                cnt[0] += 1
                return ins.then_inc(s_g)

            def gate():
                vector.wait_ge(s_g, cnt[0])

            step(vector.memset(partials[:], 0.0))
            # ---- GIoU over matched pairs ----
            vector.wait_ge(s_aux, 16)
            c_p, wh_p = mpred_t[:, 0:2], mpred_t[:, 2:4]
            c_t, wh_t = mtgt_t[:, 1:3], mtgt_t[:, 3:5]
            step(vector.scalar_tensor_tensor(p1[:], wh_p, -0.5, c_p,
                                             op0=OP.mult, op1=OP.add))
            step(vector.scalar_tensor_tensor(p2[:], wh_p, 0.5, c_p,
                                             op0=OP.mult, op1=OP.add))
            step(vector.scalar_tensor_tensor(t1[:], wh_t, -0.5, c_t,
                                             op0=OP.mult, op1=OP.add))
            step(vector.scalar_tensor_tensor(t2[:], wh_t, 0.5, c_t,
                                             op0=OP.mult, op1=OP.add))
            gate()
            step(vector.tensor_tensor(imin[:], p2[:], t2[:], OP.min))
            step(vector.tensor_tensor(imax[:], p1[:], t1[:], OP.max))
            gate()
            step(vector.scalar_tensor_tensor(dd[:], imax[:], -1.0, imin[:],
                                             op0=OP.mult, op1=OP.add))
            gate()
            step(vector.tensor_scalar_max(dr[:], dd[:], 0.0))
            gate()
            step(vector.tensor_mul(inter[:], dr[:, 0:1], dr[:, 1:2]))
            step(vector.tensor_mul(areas[:, 0:1], mpred_t[:, 2:3],
                                   mpred_t[:, 3:4]))
            step(vector.tensor_mul(areas[:, 1:2], mtgt_t[:, 3:4],
                                   mtgt_t[:, 4:5]))
            gate()
            step(vector.tensor_add(pta[:], areas[:, 0:1], areas[:, 1:2]))
            gate()
            step(vector.scalar_tensor_tensor(union[:], inter[:], -1.0, pta[:],
                                             op0=OP.mult, op1=OP.add))
            gate()
            step(vector.tensor_scalar_add(ueps[:], union[:], EPS))
            gate()
            step(vector.reciprocal(runi[:], ueps[:]))
            gate()
            step(vector.tensor_mul(iou[:], inter[:], runi[:]))
            step(vector.tensor_tensor(cmax[:], p2[:], t2[:], OP.max))
            step(vector.tensor_tensor(cmin[:], p1[:], t1[:], OP.min))
            gate()
            step(vector.scalar_tensor_tensor(dc[:], cmin[:], -1.0, cmax[:],
                                             op0=OP.mult, op1=OP.add))
            gate()
            step(vector.tensor_mul(conv[:], dc[:, 0:1], dc[:, 1:2]))
            gate()
            step(vector.scalar_tensor_tensor(cmu[:], union[:], -1.0, conv[:],
                                             op0=OP.mult, op1=OP.add))
            step(vector.tensor_scalar_add(ceps[:], conv[:], EPS))
            gate()
            step(vector.reciprocal(rcon[:], ceps[:]))
            gate()
            step(vector.tensor_mul(term[:], cmu[:], rcon[:]))
            gate()
            step(vector.scalar_tensor_tensor(gfin[:], term[:], -1.0, iou[:],
                                             op0=OP.mult, op1=OP.add,
                                             accum_out=partials[0:P, 7:8]))
            # ---- focal, ACT-independent part ----
            step(vector.tensor_scalar(oh[:], iota_t[:], mtgt_t[:, 0:1], None,
                                      OP.is_equal))
            step(vector.tensor_scalar(q12[:], mpred_t[:, 5:CH], -2.0, 1.0,
                                      OP.mult, OP.add))
            gate()
            step(vector.tensor_mul(sel2[:], oh[:], q12[:]))
            gate()
            step(vector.tensor_add(om[:], mpred_t[:, 5:CH], sel2[:]))
            gate()
            step(vector.tensor_mul(sq[:], om[:], om[:]))
            # ---- focal, ACT-consuming part ----
            vector.wait_ge(s_act, 1)
            step(vector.tensor_sub(df[:], lp[:], l1p[:]))
            gate()
            step(vector.tensor_mul(sel[:], oh[:], df[:]))
            gate()
            step(vector.tensor_add(s[:], sel[:], l1p[:]))
            gate()
            vector.scalar_tensor_tensor(
                foc[:], sq[:], 0.0, s[:], op0=OP.add, op1=OP.mult,
                accum_out=partials[0:P, 6:7]).then_inc(s_dve)

        @block.scalar
        def _(scalar):
            # descriptor generation first — no data deps, so the ring
            # transfers start as early as possible
            scalar.dma_start(auxt[:], aux.ap()).then_inc(s_aux, 16)
            with nc.allow_non_contiguous_dma(reason="strided obj column"):
                scalar.dma_start(col1[:, RA:, :],
                                 src1[:, RA:, :]).then_inc(s_col1b, 16)
            # prime the Ln table set (input is partials[0,0]==0 after the
            # memset; scale=0, bias=1 -> Ln(1) = 0, data-independent); the
            # ~1.3us table load hides under the ring transfers
            scalar.wait_ge(s_g, 1)
            scalar.activation(prime[:], partials[0:1, 0:1], AF.Ln,
                              bias=1.0, scale=0.0)
            # matched objectness + focal logs
            scalar.wait_ge(s_aux, 16)
            scalar.activation(partials[0:P, 4:5], mpred_t[:, 4:5], AF.Ln)
            scalar.activation(partials[0:P, 5:6], mpred_t[:, 4:5], AF.Ln,
                              bias=1.0, scale=-1.0)
            scalar.activation(lp[:], mpred_t[:, 5:CH], AF.Ln)
            scalar.activation(l1p[:], mpred_t[:, 5:CH], AF.Ln,
                              bias=1.0, scale=-1.0).then_inc(s_act)
            # full-N objectness scan
            scalar.wait_ge(s_col0, 16)
            scalar.activation(objt0[:], col0[:], AF.Ln, bias=1.0, scale=-1.0,
                              accum_out=partials[0:ROWS_P, 1:2]).then_inc(s_obj)
            scalar.wait_ge(s_col1a, 16)
            scalar.wait_ge(s_col1b, 16)
            scalar.activation(objt1[:], col1[:], AF.Ln, bias=1.0, scale=-1.0,
                              accum_out=partials[0:ROWS_P, 0:1]).then_inc(s_obj)
            # final dump: only obj1's column (120 x 4B); the rest went
            # out early on the sync ring
            with nc.allow_non_contiguous_dma(reason="single column out"):
                scalar.dma_start(out.ap()[:, 0:1],
                                 partials[:, 0:1]).then_inc(s_out, 16)
            scalar.wait_ge(s_out, 16)


    return nc


def _get_built():
    global _BUILT
    if _BUILT is None:
        _BUILT = _build()
    return _BUILT


LAST_RESULTS = None

_IOTA = np.ascontiguousarray(
    np.tile(np.arange(NUM_CLASSES, dtype=np.float32), (P, 1)))


def _make_in_maps(preds, targets, rows, cols):
    in_maps = []
    bidx = np.arange(BL)[:, None]
    for k in range(NCORES):
        b0 = k * BL
        pl = preds[b0:b0 + BL]
        tl = targets[b0:b0 + BL]
        mp = pl[bidx, rows[b0:b0 + BL]].reshape(P, CH)
        mt = tl[bidx, cols[b0:b0 + BL]].reshape(P, 5)
        in_maps.append({
            "preds": np.ascontiguousarray(pl),
            "aux": np.ascontiguousarray(
                np.concatenate([mp, mt, _IOTA], axis=1)),
        })
    return in_maps


def _combine(acc):
    objfull = acc[0] + acc[1] + acc[2] + acc[3]   # sum log(1-obj) over all B,N
    ml, ml1, F, G = acc[4], acc[5], acc[6], acc[7]
    box = 1.0 - G / (B * T)
    obj = -ml / (B * T) + (ml1 - objfull) / (B * (N - T))
    cls = -ALPHA * F / (B * T * NUM_CLASSES)
    total = box + obj + cls
    return (np.float32(total), np.float32(box), np.float32(obj),
            np.float32(cls))


def _ensure_axon_hooks_importable():
    """concourse.bass_utils imports antenv.axon_hooks when tracing is
    requested via env; some images lack that module.  Register a stub that
    reports no hook so the run degrades to trace-less instead of crashing."""
    try:
        import antenv  # noqa: F401
        import antenv.axon_hooks  # noqa: F401
    except ImportError:
        import sys
        import types
        try:
            import antenv
        except ImportError:
            return
        mod = types.ModuleType("antenv.axon_hooks")
        mod.get_axon_ntff_profile_hook = lambda: None
        mod.set_axon_ntff_profile_hook = lambda h: None
        sys.modules.setdefault("antenv.axon_hooks", mod)
        if not hasattr(antenv, "axon_hooks"):
            antenv.axon_hooks = mod


def kernel(preds, targets):
    global LAST_RESULTS
    _ensure_axon_hooks_importable()
    preds = np.ascontiguousarray(np.asarray(preds, dtype=np.float32))
    targets = np.ascontiguousarray(np.asarray(targets, dtype=np.float32))
    rows, cols = _match(np.asarray(preds[..., :4], dtype=np.float32),
                        np.asarray(targets[..., 1:5], dtype=np.float32))

    nc = _get_built()
    in_maps = _make_in_maps(preds, targets, rows, cols)

    from concourse.bass_utils import run_bass_kernel_spmd
    res = run_bass_kernel_spmd(nc, in_maps, core_ids=list(range(NCORES)))
    LAST_RESULTS = res

    acc = np.zeros(NCOL, dtype=np.float64)
    for r in res.results:
        acc += r["out"].astype(np.float64).sum(axis=0)
    return _combine(acc)


# revision 44
# speedup vs baseline: 1.0092x; 1.0092x over previous
"""DetectionLoss Trainium2 kernel.

Host side (mirrors the reference, which also does this on CPU): per-image
Hungarian matching on the L1 box cost, plus gathering the 30 matched
pred/target rows per image that the matching selects.  Device side
(Bass/Tile, 8 NeuronCores, data-parallel over the batch — 2 images per
core): all loss arithmetic — the full (B,N) objectness BCE scan (strided
DMA of the objectness column), GIoU over matched pairs, focal
classification loss — reduced to per-core partial sums.  Host sums the 8
partial-sum vectors and forms the three scalar losses.
"""

import numpy as np

NUM_CLASSES = 80
ALPHA, GAMMA = 0.25, 2.0
EPS = 1e-07

B, N, T, CH = 16, 8400, 30, 85  # batch, anchors, targets, channels (5+C)
NCORES = 8
BL = B // NCORES          # images per core
P = BL * T                # matched pairs per core (partition dim)
ROWS_P = 120              # partitions for the obj scan (8400 = 120*70)
ROWS_F = N // ROWS_P      # 70 rows per partition
NCOL = 8                  # partials columns


# ---------------------------------------------------------------- host match
def _lsa(cost):
    """Min-cost rectangular linear sum assignment (verbatim from reference)."""
    cost = np.asarray(cost, dtype=np.float64)
    transposed = cost.shape[0] > cost.shape[1]
    C = cost.T if transposed else cost
    n, m = C.shape
    u = np.zeros(n + 1)
    v = np.zeros(m + 1)
    p = np.zeros(m + 1, dtype=np.int64)
    way = np.zeros(m + 1, dtype=np.int64)
    Cp = np.zeros((n + 1, m + 1))
    Cp[1:, 1:] = C
    for i in range(1, n + 1):
        p[0] = i
        j0 = 0
        minv = np.full(m + 1, np.inf)
        used = np.zeros(m + 1, dtype=bool)
        while True:
            used[j0] = True
            i0 = p[j0]
            js = np.nonzero(~used)[0]
            cur = Cp[i0, js] - u[i0] - v[js]
            better = cur < minv[js]
            minv[js] = np.where(better, cur, minv[js])
            way[js[better]] = j0
            k = int(np.argmin(minv[js]))
            delta = minv[js][k]
            j1 = int(js[k])
            u[p[used]] += delta
            v[used] -= delta
            minv[js] -= delta
            j0 = j1
            if p[j0] == 0:
                break
        while j0:
            j1 = way[j0]
            p[j0] = p[j1]
            j0 = j1
    rows = p[1:] - 1
    cols = np.arange(m)
    mask = rows >= 0
    r, c = rows[mask], cols[mask]
    if transposed:
        r, c = c, r
    order = np.argsort(r)
    return r[order], c[order]


def _match(pred_boxes_np, target_boxes_np):
    Bn, _, _ = pred_boxes_np.shape
    Tn = target_boxes_np.shape[1]
    rows = np.zeros((Bn, Tn), dtype=np.int64)
    cols = np.zeros((Bn, Tn), dtype=np.int64)
    for b in range(Bn):
        cost = np.abs(
            pred_boxes_np[b][:, None, :] - target_boxes_np[b][None, :, :]
        ).sum(-1)
        r, c = _lsa(cost)
        rows[b], cols[b] = r, c
    return rows, cols


# ---------------------------------------------------------------- device kernel
_BUILT = None


def _build():
    """Raw-Bass (no Tile) build: three engines (sync, scalar, vector) with a
    hand-rolled semaphore plan — avoids the Tile entry barrier and exit
    butterfly so the column DMAs start right after engine bring-up.

    Raw-bass DVE ops have no implicit same-engine interlock (a distance-1
    consumer can read SBUF before the producer's write lands — verified on
    HW), so every dependent DVE pair is gated on a self-semaphore.
    """
    import concourse.bass as bass
    from concourse import mybir

    f32 = mybir.dt.float32
    AF = mybir.ActivationFunctionType
    OP = mybir.AluOpType

    nc = bass.Bass("TRN2", target_bir_lowering=False)

    preds = nc.declare_dram_parameter("preds", [BL, N, CH], f32, isOutput=False)
    # aux = [matched pred rows (85) | matched tgt rows (5) | iota (80)]
    AUXW = CH + 5 + NUM_CLASSES
    aux = nc.declare_dram_parameter("aux", [P, AUXW], f32, isOutput=False)
    out = nc.declare_dram_parameter("out", [128, NCOL], f32, isOutput=True)

    # ring split of image 1's objectness column (sync ~957 el/us since it
    # starts ~2us earlier than the scalar ring, which also carries aux)
    RA = 10
    Cc = NUM_CLASSES

    partials = nc.alloc_sbuf_tensor("k_partials", [128, NCOL], f32)
    auxt = nc.alloc_sbuf_tensor("k_auxt", [P, AUXW], f32)
    col0 = nc.alloc_sbuf_tensor("k_col0", [ROWS_P, ROWS_F, 1], f32)
    col1 = nc.alloc_sbuf_tensor("k_col1", [ROWS_P, ROWS_F, 1], f32)
    objt0 = nc.alloc_sbuf_tensor("k_objt0", [ROWS_P, ROWS_F, 1], f32)
    objt1 = nc.alloc_sbuf_tensor("k_objt1", [ROWS_P, ROWS_F, 1], f32)
    prime = nc.alloc_sbuf_tensor("k_prime", [1, 1], f32)

    def g2(name):
        return nc.alloc_sbuf_tensor("k_" + name, [P, 2], f32)

    def g1(name):
        return nc.alloc_sbuf_tensor("k_" + name, [P, 1], f32)

    def gc(name):
        return nc.alloc_sbuf_tensor("k_" + name, [P, Cc], f32)

    p1, p2, t1, t2 = g2("p1"), g2("p2"), g2("t1"), g2("t2")
    imin, imax, dd, dr = g2("imin"), g2("imax"), g2("dd"), g2("dr")
    areas, cmax, cmin, dc = g2("areas"), g2("cmax"), g2("cmin"), g2("dc")
    inter, pta, union = g1("inter"), g1("pta"), g1("union")
    ueps, runi, iou = g1("ueps"), g1("runi"), g1("iou")
    conv, cmu, ceps, rcon, term = (g1("conv"), g1("cmu"), g1("ceps"),
                                   g1("rcon"), g1("term"))
    gfin = g1("gfin")
    oh, lp, l1p, df, sel, s = (gc("oh"), gc("lp"), gc("l1p"), gc("df"),
                               gc("sel"), gc("s"))
    q12, sel2, om, sq, foc = (gc("q12"), gc("sel2"), gc("om"), gc("sq"),
                              gc("foc"))

    mpred_t = auxt[:, 0:CH]
    mtgt_t = auxt[:, CH:CH + 5]
    iota_t = auxt[:, CH + 5:AUXW]

    src_all = preds.ap().rearrange("b (p r) c -> (b p) r c", p=ROWS_P)
    src0 = src_all[0:ROWS_P, :, 4:5]
    src1 = src_all[ROWS_P:2 * ROWS_P, :, 4:5]

    s_aux = nc.alloc_semaphore("s_aux")
    s_col0 = nc.alloc_semaphore("s_col0")
    s_col1a = nc.alloc_semaphore("s_col1a")
    s_col1b = nc.alloc_semaphore("s_col1b")
    s_out = nc.alloc_semaphore("s_out")
    s_act = nc.alloc_semaphore("s_act")
    s_obj = nc.alloc_semaphore("s_obj")
    s_dve = nc.alloc_semaphore("s_dve")
    s_g = nc.alloc_semaphore("s_g")

    with nc.Block() as block:

        @block.sync
        def _(sync):
            # objectness column element-gathers on the SP HWDGE ring;
            # no waits — start as soon as the engine is up.
            with nc.allow_non_contiguous_dma(reason="strided obj column"):
                sync.dma_start(col0[:], src0).then_inc(s_col0, 16)
                sync.dma_start(col1[:, 0:RA, :],
                               src1[:, 0:RA, :]).then_inc(s_col1a, 16)


        @block.vector
        def _(vector):
            cnt = [0]

            def step(ins):
                cnt[0] += 1
                return ins.then_inc(s_g)

            def gate():
                vector.wait_ge(s_g, cnt[0])

            step(vector.memset(partials[:], 0.0))
            # ---- GIoU over matched pairs ----
            vector.wait_ge(s_aux, 16)
            c_p, wh_p = mpred_t[:, 0:2], mpred_t[:, 2:4]
            c_t, wh_t = mtgt_t[:, 1:3], mtgt_t[:, 3:5]
            step(vector.scalar_tensor_tensor(p1[:], wh_p, -0.5, c_p,
                                             op0=OP.mult, op1=OP.add))
            step(vector.scalar_tensor_tensor(p2[:], wh_p, 0.5, c_p,
                                             op0=OP.mult, op1=OP.add))
            step(vector.scalar_tensor_tensor(t1[:], wh_t, -0.5, c_t,
                                             op0=OP.mult, op1=OP.add))
            step(vector.scalar_tensor_tensor(t2[:], wh_t, 0.5, c_t,
                                             op0=OP.mult, op1=OP.add))
            gate()
            step(vector.tensor_tensor(imin[:], p2[:], t2[:], OP.min))
            step(vector.tensor_tensor(imax[:], p1[:], t1[:], OP.max))
            gate()
            step(vector.scalar_tensor_tensor(dd[:], imax[:], -1.0, imin[:],
                                             op0=OP.mult, op1=OP.add))
            gate()
            step(vector.tensor_scalar_max(dr[:], dd[:], 0.0))
            gate()
            step(vector.tensor_mul(inter[:], dr[:, 0:1], dr[:, 1:2]))
            step(vector.tensor_mul(areas[:, 0:1], mpred_t[:, 2:3],
                                   mpred_t[:, 3:4]))
            step(vector.tensor_mul(areas[:, 1:2], mtgt_t[:, 3:4],
                                   mtgt_t[:, 4:5]))
            gate()
            step(vector.tensor_add(pta[:], areas[:, 0:1], areas[:, 1:2]))
            gate()
            step(vector.scalar_tensor_tensor(union[:], inter[:], -1.0, pta[:],
                                             op0=OP.mult, op1=OP.add))
            gate()
            step(vector.tensor_scalar_add(ueps[:], union[:], EPS))
            gate()
            step(vector.reciprocal(runi[:], ueps[:]))
            gate()
            step(vector.tensor_mul(iou[:], inter[:], runi[:]))
            step(vector.tensor_tensor(cmax[:], p2[:], t2[:], OP.max))
            step(vector.tensor_tensor(cmin[:], p1[:], t1[:], OP.min))
            gate()
            step(vector.scalar_tensor_tensor(dc[:], cmin[:], -1.0, cmax[:],
                                             op0=OP.mult, op1=OP.add))
            gate()
            step(vector.tensor_mul(conv[:], dc[:, 0:1], dc[:, 1:2]))
            gate()
            step(vector.scalar_tensor_tensor(cmu[:], union[:], -1.0, conv[:],
                                             op0=OP.mult, op1=OP.add))
            step(vector.tensor_scalar_add(ceps[:], conv[:], EPS))
            gate()
            step(vector.reciprocal(rcon[:], ceps[:]))
            gate()
            step(vector.tensor_mul(term[:], cmu[:], rcon[:]))
            gate()
            step(vector.scalar_tensor_tensor(gfin[:], term[:], -1.0, iou[:],
                                             op0=OP.mult, op1=OP.add,
                                             accum_out=partials[0:P, 7:8]))
            # ---- focal, ACT-independent part ----
            step(vector.tensor_scalar(oh[:], iota_t[:], mtgt_t[:, 0:1], None,
                                      OP.is_equal))
            step(vector.tensor_scalar(q12[:], mpred_t[:, 5:CH], -2.0, 1.0,
                                      OP.mult, OP.add))
            gate()
            step(vector.tensor_mul(sel2[:], oh[:], q12[:]))
            gate()
            step(vector.tensor_add(om[:], mpred_t[:, 5:CH], sel2[:]))
            gate()
            step(vector.tensor_mul(sq[:], om[:], om[:]))
            # ---- focal, ACT-consuming part ----
            vector.wait_ge(s_act, 1)
            step(vector.tensor_sub(df[:], lp[:], l1p[:]))
            gate()
            step(vector.tensor_mul(sel[:], oh[:], df[:]))
            gate()
            step(vector.tensor_add(s[:], sel[:], l1p[:]))
            gate()
            vector.scalar_tensor_tensor(
                foc[:], sq[:], 0.0, s[:], op0=OP.add, op1=OP.mult,
                accum_out=partials[0:P, 6:7]).then_inc(s_dve)

        @block.scalar
        def _(scalar):
            # descriptor generation first — no data deps, so the ring
            # transfers start as early as possible
            scalar.dma_start(auxt[:], aux.ap()).then_inc(s_aux, 16)
            with nc.allow_non_contiguous_dma(reason="strided obj column"):
                scalar.dma_start(col1[:, RA:, :],
                                 src1[:, RA:, :]).then_inc(s_col1b, 16)
            # prime the Ln table set (input is partials[0,0]==0 after the
            # memset; scale=0, bias=1 -> Ln(1) = 0, data-independent); the
            # ~1.3us table load hides under the ring transfers
            scalar.wait_ge(s_g, 1)
            scalar.activation(prime[:], partials[0:1, 0:1], AF.Ln,
                              bias=1.0, scale=0.0)
            # matched objectness + focal logs
            scalar.wait_ge(s_aux, 16)
            scalar.activation(partials[0:P, 4:5], mpred_t[:, 4:5], AF.Ln)
            scalar.activation(partials[0:P, 5:6], mpred_t[:, 4:5], AF.Ln,
                              bias=1.0, scale=-1.0)
            scalar.activation(lp[:], mpred_t[:, 5:CH], AF.Ln)
            scalar.activation(l1p[:], mpred_t[:, 5:CH], AF.Ln,
                              bias=1.0, scale=-1.0).then_inc(s_act)
            # full-N objectness scan
            scalar.wait_ge(s_col0, 16)
            scalar.activation(objt0[:], col0[:], AF.Ln, bias=1.0, scale=-1.0,
                              accum_out=partials[0:ROWS_P, 0:1]).then_inc(s_obj)
            scalar.wait_ge(s_col1a, 16)
            scalar.wait_ge(s_col1b, 16)
            scalar.activation(objt1[:], col1[:], AF.Ln, bias=1.0, scale=-1.0,
                              accum_out=partials[0:ROWS_P, 1:2]).then_inc(s_obj)
            # dump partials straight from this engine (its ring drained
            # first and no cross-engine semaphore hop is needed)
            scalar.wait_ge(s_dve, 1)
            scalar.dma_start(out.ap(), partials[:]).then_inc(s_out, 16)
            scalar.wait_ge(s_out, 16)


    return nc


def _get_built():
    global _BUILT
    if _BUILT is None:
        _BUILT = _build()
    return _BUILT


LAST_RESULTS = None

_IOTA = np.ascontiguousarray(
    np.tile(np.arange(NUM_CLASSES, dtype=np.float32), (P, 1)))


def _make_in_maps(preds, targets, rows, cols):
    in_maps = []
    bidx = np.arange(BL)[:, None]
    for k in range(NCORES):
        b0 = k * BL
        pl = preds[b0:b0 + BL]
        tl = targets[b0:b0 + BL]
        mp = pl[bidx, rows[b0:b0 + BL]].reshape(P, CH)
        mt = tl[bidx, cols[b0:b0 + BL]].reshape(P, 5)
        in_maps.append({
            "preds": np.ascontiguousarray(pl),
            "aux": np.ascontiguousarray(
                np.concatenate([mp, mt, _IOTA], axis=1)),
        })
    return in_maps


def _combine(acc):
    objfull = acc[0] + acc[1] + acc[2] + acc[3]   # sum log(1-obj) over all B,N
    ml, ml1, F, G = acc[4], acc[5], acc[6], acc[7]
    box = 1.0 - G / (B * T)
    obj = -ml / (B * T) + (ml1 - objfull) / (B * (N - T))
    cls = -ALPHA * F / (B * T * NUM_CLASSES)
    total = box + obj + cls
    return (np.float32(total), np.float32(box), np.float32(obj),
            np.float32(cls))


def _ensure_axon_hooks_importable():
    """concourse.bass_utils imports antenv.axon_hooks when tracing is
    requested via env; some images lack that module.  Register a stub that
    reports no hook so the run degrades to trace-less instead of crashing."""
    try:
        import antenv  # noqa: F401
        import antenv.axon_hooks  # noqa: F401
    except ImportError:
        import sys
        import types
        try:
            import antenv
        except ImportError:
            return
        mod = types.ModuleType("antenv.axon_hooks")
        mod.get_axon_ntff_profile_hook = lambda: None
        mod.set_axon_ntff_profile_hook = lambda h: None
        sys.modules.setdefault("antenv.axon_hooks", mod)
        if not hasattr(antenv, "axon_hooks"):
            antenv.axon_hooks = mod


def kernel(preds, targets):
    global LAST_RESULTS
    _ensure_axon_hooks_importable()
    preds = np.ascontiguousarray(np.asarray(preds, dtype=np.float32))
    targets = np.ascontiguousarray(np.asarray(targets, dtype=np.float32))
    rows, cols = _match(np.asarray(preds[..., :4], dtype=np.float32),
                        np.asarray(targets[..., 1:5], dtype=np.float32))

    nc = _get_built()
    in_maps = _make_in_maps(preds, targets, rows, cols)

    from concourse.bass_utils import run_bass_kernel_spmd
    res = run_bass_kernel_spmd(nc, in_maps, core_ids=list(range(NCORES)))
    LAST_RESULTS = res

    acc = np.zeros(NCOL, dtype=np.float64)
    for r in res.results:
        acc += r["out"].astype(np.float64).sum(axis=0)
    return _combine(acc)
